# revision 98
# baseline (speedup 1.0000x reference)
"""Single-head causal attention (B=4, S=4096, E=512, DK=DV=64) on 8 trn2 cores.

Sharding: 2 cores per batch element, balanced causal split at 256-row
granularity. Each core owns 8 q-groups of 256 rows at positions
[512g+256, 512g+512). Role A (even cores) holds emb in natural token order so
those positions are its own q rows; role B's host-side embT swaps the two
256-halves of every 512-token block, which lands its q rows (orig
[512g, 512g+256)) at the same static positions while keeping every causal key
inside the group's static 4(g+1)-key-tile prefix. Both roles run the identical
SPMD program; the only per-role differences are host data (embT layout, mask
slab for the last quad's first two tiles: keep for A, zero for B).

Everything is computed transposed (d-major) until PV. Scores use fp8e4m3
DoubleRow matmuls (2x PE throughput; the second contraction row is zero).
PV is "flipped": the exp'd score tile P^T is the stationary operand and the
ones-augmented V (fp8, token-natural) is the moving operand, so the output
comes out token-major — no finalize transposes — and each 256-key pair costs
one 66-wide DoubleRow matmul. Group 0 (rows < 512, where softmax mass
concentrates on few keys) runs PV in bf16 for precision. The softmax column
sum rides along as a ones column of V.

The Activation engine's exp stream ([128, 1024] quads, ~37us busy) is the
critical spine; everything else is scheduled around keeping it saturated:
masked quads run second in each group so their DVE mask-mul overlaps later
exps; PV quads are emitted two positions late so a parked PV can never
head-of-line-block the next scores in the 4-deep PE wait queue (each matmul
is an Ldweights+Matmult pair, so only 2 pending matmuls fit); the next
group's first scores is hoisted into the last exp slot behind only q_proj;
warm-up zero-matmuls ramp the PE p-state while the first DMAs fly; and the
weights + first q-half land in a single leading DMA.
"""

import sys

for _p in ("/opt/trn_rl_repo",):
    if _p not in sys.path:
        sys.path.insert(0, _p)

import numpy as np
import ml_dtypes

import concourse.bass as bass
import concourse.bacc as bacc
import concourse.mybir as mybir
from concourse.bass_utils import run_bass_kernel_spmd
from concourse.tile import TileContext

B, S, E, DK, DV = 4, 4096, 512, 64, 64
P = 128
NCORES = 8
NG = 8          # attention q-groups per core
QG = 256        # q rows per group
KG = 512        # tokens per kv/projection group
NJ = S // KG    # kv groups (8)
EC = E // P     # embedding chunks (4)
F32 = mybir.dt.float32
BF16 = mybir.dt.bfloat16
F8 = mybir.dt.float8e4
DR = mybir.MatmulPerfMode.DoubleRow
VW = DV + 2     # V columns + ones column + pad (66)

# scheduling knobs (sweepable via build_program(cfg))
CFG = {
    "pv_defer": 2,       # PV quads deferred this many positions
    "pt_bufs": 8,
    "kp_slot": 1,        # position for KP(g+1) filler in groups >= 1
    "vn_slot": 2,        # position for VN(g+1)
    "qp_slot": 0,        # position for QP(g+1) ("pre" = just before hoist)
    "split_g0": True,    # split group 0's exp into two tile-pair halves
    "warmup": 5,        # dummy PE matmuls to ramp the clock before data lands
    # (group, position) quads whose exp runs as a Schraudolph bit-trick on
    # DVE (+fp8 convert on Pool) instead of the saturated Act engine
    "sch": ((7, 6), (7, 4), (6, 1), (5, 1), (6, 6)),
    "sch_pool": False,   # fp8 convert step on Pool (False: DVE)
    # groups whose k8/q8 conversions run on the Act engine (idle during the
    # DMA-bound early phase; Identity shares Exp's act table set, no reload)
    "conv_act_upto": -1,
}

SCH_A = float(0.125 * (1 << 23) / np.log(2))
SCH_B = float((127 << 23) - 366500)


def build_program(cfg=None):
    cfg = {**CFG, **(cfg or {})}
    nc = bacc.Bacc("TRN2", target_bir_lowering=False, debug=False, num_devices=NCORES)

    embT = nc.declare_dram_parameter("embT", [E, S], BF16, isOutput=False)
    # one leading DMA: per partition p, [wqkv rows c*128+p (4x256 cols) |
    # embT rows c*128+p, tokens 256:512 (4x256)] — weights + the et0 half
    # that feeds q_proj(0) land together in a single HWDGE slot
    pre = nc.declare_dram_parameter("pre", [P, 2 * EC * 2 * P], BF16, isOutput=False)
    # col 0: [bk; bv], col 1 rows 0:64: bq
    bkv = nc.declare_dram_parameter("bkv", [P, 2], F32, isOutput=False)
    # cols 0:1024 mask (last-quad slab), 1024:1152 ones, 1152:1216 bv row
    cst = nc.declare_dram_parameter("cst", [P, 4 * QG + P + DV], BF16, isOutput=False)
    out = nc.declare_dram_parameter("out", [NG, QG, DV], F32, isOutput=True)

    with TileContext(nc) as tc:
        with (
            tc.tile_pool(name="singles", bufs=1) as singles,
            tc.tile_pool(name="pt", bufs=cfg["pt_bufs"]) as pt_pool,
            tc.tile_pool(name="pt0", bufs=1) as pt0_pool,
            tc.tile_pool(name="fin", bufs=2) as fin_pool,
            tc.tile_pool(name="sch", bufs=2) as sch_pool_t,
            tc.tile_pool(name="res", bufs=2) as res_pool,
            tc.tile_pool(name="ps_s", bufs=2, space="PSUM") as ps_pool,
            tc.tile_pool(name="ps_o", bufs=1, space="PSUM") as po_pool,
            tc.tile_pool(name="ps_sm", bufs=2, space="PSUM") as sm_pool,
        ):
            # ---- persistent tensors ----
            et = [
                singles.tile([P, EC, KG], BF16, name=f"et{j}") for j in range(NJ)
            ]
            k8 = singles.tile([DK, 2, S], F8)            # K^T, j=1 zeros
            q8 = singles.tile([DK, 2, NG * QG], F8)      # Q^T, j=1 zeros
            vt = singles.tile([P, S], BF16)              # rows 64:128 = V^T
            vn8 = singles.tile([P, S // P, VW], F8)      # V natural + ones + pad
            vnb = singles.tile([P, 4, VW], BF16)         # bf16 V for group 0
            pre_sb = singles.tile([P, 2, EC, 2 * P], BF16)
            w_sb = pre_sb[:, 0]                          # [P, EC, 256]
            et0b = pre_sb[:, 1]                          # [P, EC, 256] tok 256:512
            bkv_sb = singles.tile([P, 2], F32)
            cst_sb = singles.tile([P, 4 * QG + P + DV], BF16)
            msk_sb = cst_sb[:, 0:4 * QG]
            ones_row = cst_sb[0:1, 4 * QG:4 * QG + P]
            bv_row = cst_sb[0:1, 4 * QG + P:4 * QG + P + DV]
            po = po_pool.tile([P, 2, KG], F32)           # PV accum, h-halves in
                                                         # separate PSUM banks

            # ---- input DMAs, in consumption order ----
            embT_r = embT[:].rearrange("(c p) t -> p c t", p=P)

            def et_dma(j, lo=0, hi=KG):
                nc.sync.dma_start(
                    out=et[j][:, :, lo:hi], in_=embT_r[:, :, KG * j + lo:KG * j + hi]
                )

            # biases ride the GPSIMD/SWDGE path: no HWDGE slot at all, so the
            # embedding stream's generation pipeline is untouched
            nc.gpsimd.dma_start(out=bkv_sb, in_=bkv[:])
            nc.sync.dma_start(
                out=pre_sb.rearrange("p a b c -> p (a b c)"), in_=pre[:]
            )
            et_dma(0, 0, QG)
            # q-halves first: q_proj(g) needs only tokens [QG, KG) of et[g]
            et_dma(1, QG, KG)
            et_dma(1, 0, QG)
            nc.sync.dma_start(out=cst_sb, in_=cst[:])
            et_dma(2, QG, KG)
            et_dma(2, 0, QG)
            et_dma(3, QG, KG)
            et_dma(3, 0, QG)
            et_dma(4)
            et_dma(5)
            et_dma(6)
            et_dma(7)

            # PE p-state warm-up: the tensor engine only reaches full clock
            # after 3us of continuous execution. Zero-matmuls on a scratch
            # tile ramp it while the first embedding DMAs are in flight.
            if cfg["warmup"]:
                wu = singles.tile([P, KG], BF16)
                nc.vector.memset(wu, 0.0)
                # scratch accumulator: po's h=0 bank (its first real PV
                # starts with start=True, overwriting the warmup garbage)
                for _ in range(cfg["warmup"]):
                    nc.tensor.matmul(
                        po[:, 0, :], wu[:, 0:P], wu, start=True, stop=True
                    )

            # DoubleRow j=1 contraction rows are zero. The slices group 0's
            # first scores need go on DVE (idle until the conversions start);
            # the bulk goes on GPSIMD where the multi-us memsets cost nothing.
            nc.vector.memset(k8[:, 1, 0:KG], 0.0)
            nc.vector.memset(q8[:, 1, 0:QG], 0.0)
            nc.gpsimd.memset(k8[:, 1, KG:S], 0.0)
            nc.gpsimd.memset(q8[:, 1, QG:NG * QG], 0.0)
            nc.vector.memset(vn8[:, :, DV:DV + 1], 1.0)
            nc.vector.memset(vn8[:, :, DV + 1:DV + 2], 0.0)
            nc.vector.memset(vnb[:, :, DV:DV + 1], 1.0)
            nc.vector.memset(vnb[:, :, DV + 1:DV + 2], 0.0)

            # ---- building blocks ----
            def conv_add(out_ap, in_ap, bias_ap, act):
                if act:
                    nc.scalar.activation(
                        out_ap, in_ap, mybir.ActivationFunctionType.Identity,
                        bias=bias_ap,
                    )
                else:
                    nc.vector.tensor_scalar_add(out_ap, in_ap, bias_ap)

            def kv_proj(j, lo=0, hi=KG, act=False):
                pkv = sm_pool.tile([P, KG], F32, tag="sm")
                for c in range(EC):
                    nc.tensor.matmul(
                        pkv[:, lo:hi], w_sb[:, c, 0:P], et[j][:, c, lo:hi],
                        start=(c == 0), stop=(c == EC - 1),
                    )
                conv_add(
                    k8[:, 0, KG * j + lo:KG * j + hi], pkv[0:DK, lo:hi],
                    bkv_sb[0:DK, 0:1], act
                )

            def vnat(j):
                # V projected straight into token-natural layout: the moving
                # side is Wv (64 wide) so each 128-token tile costs 64 rows
                # per chunk; stationary embedding reloads are free. The bias
                # rides as a rank-1 ones-row matmul; one DVE conversion total.
                pvn = sm_pool.tile([P, 4, DV], F32, tag="sm")
                for s in range(4):
                    for c in range(EC):
                        if j == 0 and s >= 2:
                            lh = et0b[:, c, (s - 2) * P:(s - 1) * P]
                        else:
                            lh = et[j][:, c, s * P:(s + 1) * P]
                        nc.tensor.matmul(
                            pvn[:, s, :], lh, w_sb[:, c, DK:P],
                            start=(c == 0), stop=False,
                        )
                    nc.tensor.matmul(
                        pvn[:, s, :], ones_row, bv_row, start=False, stop=True
                    )
                nc.vector.tensor_copy(vn8[:, 4 * j:4 * j + 4, 0:DV], pvn)
                if j == 0:
                    nc.vector.tensor_copy(vnb[:, 0:4, 0:DV], pvn)

            def q_proj(g, act=False):
                pq = sm_pool.tile([DK, QG], F32, tag="sm")
                qsrc = et0b if g == 0 else et[g][:, :, QG:2 * QG]
                for c in range(EC):
                    nc.tensor.matmul(
                        pq, w_sb[:, c, P:P + DK], qsrc[:, c, :],
                        start=(c == 0), stop=(c == EC - 1),
                    )
                conv_add(
                    q8[:, 0, QG * g:QG * (g + 1)], pq, bkv_sb[0:DK, 1:2], act
                )

            def scores(g, q, sch=False):
                qs = q8[:, :, QG * g:QG * (g + 1)]
                if sch:
                    # Schraudolph quads park their scores in the sm pool
                    # (idle at late positions) so the slower DVE consumer
                    # never slips the main ps rotation under the Act spine
                    halves = []
                    for h2 in range(2):
                        sm = sm_pool.tile([P, 2, QG], F32, tag="sm")
                        for i in range(2):
                            kt = 4 * q + 2 * h2 + i
                            nc.tensor.matmul(
                                sm[:, i, :], k8[:, :, kt * P:(kt + 1) * P], qs,
                                start=True, stop=True, perf_mode=DR,
                            )
                        halves.append(sm)
                    return tuple(halves)
                ps = ps_pool.tile([P, 4, QG], F32, tag="ps")
                for i in range(4):
                    kt = 4 * q + i
                    nc.tensor.matmul(
                        ps[:, i, :], k8[:, :, kt * P:(kt + 1) * P], qs,
                        start=True, stop=True, perf_mode=DR,
                    )
                return ps

            sch_set = set(map(tuple, cfg["sch"]))

            def attention(g, fillers=None, first_ps=None):
                fillers = dict(fillers or {})
                n_q = g + 1
                # the masked quad (q == g) runs second: its GPSIMD mask-mul
                # overlaps later quads' exp instead of sitting in the tail
                order = [0, g] + list(range(1, g)) if g > 0 else [0]
                ps_cur = first_ps if first_ps is not None else scores(g, order[0])
                nxt = None
                pending_pv = []
                for i, q in enumerate(order):
                    if g == 0 and cfg["split_g0"]:
                        # two halves, tiles 2,3 first — their projections and
                        # conversions complete first in the prologue
                        ps_lo, ps_hi = ps_cur
                        pt = pt0_pool.tile([P, 4, QG], BF16, tag="pt0")
                        nc.scalar.activation(
                            pt[:, 2:4, :], ps_hi,
                            mybir.ActivationFunctionType.Exp, scale=0.125,
                        )
                        nc.scalar.activation(
                            pt[:, 0:2, :], ps_lo,
                            mybir.ActivationFunctionType.Exp, scale=0.125,
                        )
                    elif g == 0:
                        pt = pt0_pool.tile([P, 4, QG], BF16, tag="pt0")
                        nc.scalar.activation(
                            pt, ps_cur, mybir.ActivationFunctionType.Exp,
                            scale=0.125,
                        )
                    elif (g, i) in sch_set:
                        # Schraudolph exp: i32(a*s + b) bit-cast to f32 is
                        # exp(s/8) to ~3%; trades saturated Act time for idle
                        # DVE/Pool time on late big-k groups
                        ti = sch_pool_t.tile([P, 4, QG], mybir.dt.int32, tag="ti")
                        for h2 in range(2):
                            nc.vector.tensor_scalar(
                                ti[:, 2 * h2:2 * h2 + 2, :], ps_cur[h2],
                                SCH_A, SCH_B,
                                mybir.AluOpType.mult, mybir.AluOpType.add,
                            )
                        pt = pt_pool.tile([P, 4, QG], F8, tag="pt")
                        eng = nc.gpsimd if cfg["sch_pool"] else nc.vector
                        if q == g:
                            # masked quad: the fp8 convert doubles as the
                            # mask multiply — the mask becomes free
                            eng.tensor_mul(
                                pt, ti.bitcast(F32),
                                msk_sb.rearrange("p (a b) -> p a b", b=QG),
                            )
                        else:
                            eng.tensor_copy(pt, ti.bitcast(F32))
                    else:
                        pt = pt_pool.tile([P, 4, QG], F8, tag="pt")
                        nc.scalar.activation(
                            pt, ps_cur, mybir.ActivationFunctionType.Exp,
                            scale=0.125,
                        )
                    if i + 1 < n_q:
                        ps_cur = scores(g, order[i + 1],
                                        sch=(g, i + 1) in sch_set)
                        for f in fillers.pop(i, ()):
                            f()
                    else:
                        # last slot: q_proj first (the hoisted scores reads its
                        # q8 slice — emission order is a correctness
                        # requirement), then the hoist, then the bulkier
                        # kv-side fillers so they don't delay the next group's
                        # first exp
                        for f in fillers.pop(i, ()):
                            f()
                        for f in fillers.pop("pre", ()):
                            f()
                        if g + 1 < NG:
                            nxt = scores(g + 1, 0)
                        for f in fillers.pop("post", ()):
                            f()
                    if q == g and (g, i) not in sch_set:
                        # DVE, h-split: GPSIMD's queue suffers multi-us Drain
                        # holds that start masks late and stall the PV chain
                        for h in range(2):
                            nc.vector.tensor_mul(
                                pt[:, :, P * h:P * (h + 1)],
                                pt[:, :, P * h:P * (h + 1)],
                                msk_sb.rearrange("p (a b) -> p a b", b=QG)[
                                    :, :, P * h:P * (h + 1)
                                ],
                            )

                    def mk_pv(pt, q, i):
                        def emit():
                            if g == 0:
                                for kt in range(4):
                                    for h in range(2):
                                        nc.tensor.matmul(
                                            po[:, h, 0:VW],
                                            pt[:, kt, P * h:P * (h + 1)],
                                            vnb[:, kt, :],
                                            start=(kt == 0), stop=(kt == 3),
                                        )
                            else:
                                for p2 in range(2):
                                    for h in range(2):
                                        nc.tensor.matmul(
                                            po[:, h, 0:VW],
                                            pt[:, 2 * p2:2 * p2 + 2,
                                               P * h:P * (h + 1)],
                                            vn8[:, 2 * (2 * q + p2):
                                                2 * (2 * q + p2) + 2, :],
                                            start=(i == 0 and p2 == 0),
                                            stop=(i == n_q - 1 and p2 == 1),
                                            perf_mode=DR,
                                        )
                        return emit

                    # defer PV one position: a PV quad parked on its pt fills
                    # the 4-deep PE wait queue (Ldweights+Matmult pairs) and
                    # head-of-line-blocks the next scores the Act engine needs
                    pending_pv.append(mk_pv(pt, q, i))
                    if len(pending_pv) > cfg["pv_defer"]:
                        pending_pv.pop(0)()
                for i in sorted(fillers):
                    for f in fillers[i]:
                        f()
                for f in pending_pv:
                    f()
                rs = fin_pool.tile([P, 2, 1], F32, tag="rs")
                nc.vector.reciprocal(rs, po[:, :, DV:DV + 1])
                res = res_pool.tile([P, 2, DV], F32, tag="res")
                for h in range(2):
                    nc.vector.tensor_scalar_mul(
                        res[:, h, :], po[:, h, 0:DV], rs[:, h, :]
                    )
                nc.sync.dma_start(
                    out=out[:][g].rearrange("(s p) d -> p s d", p=P), in_=res
                )
                return nxt

            # ---- emission schedule ----
            # Custom prologue: all projection matmuls first (PE), then the k8
            # conversions (DVE) ahead of the vt conversions they don't gate,
            # and group 0's scores in two independent ps tiles so each exp
            # half waits only on its own inputs.
            qs0 = q8[:, :, 0:QG]
            if cfg["split_g0"]:
                pq0 = ps_pool.tile([DK, QG], F32, tag="ps")
                for c in range(EC):
                    nc.tensor.matmul(
                        pq0, w_sb[:, c, P:P + DK], et0b[:, c, :],
                        start=(c == 0), stop=(c == EC - 1),
                    )
                pkv_b = sm_pool.tile([P, QG], F32, tag="sm")
                for c in range(EC):
                    nc.tensor.matmul(
                        pkv_b, w_sb[:, c, 0:P], et0b[:, c, :],
                        start=(c == 0), stop=(c == EC - 1),
                    )
                pkv_a = sm_pool.tile([P, QG], F32, tag="sm")
                for c in range(EC):
                    nc.tensor.matmul(
                        pkv_a, w_sb[:, c, 0:P], et[0][:, c, 0:QG],
                        start=(c == 0), stop=(c == EC - 1),
                    )
                nc.vector.tensor_scalar_add(q8[:, 0, 0:QG], pq0, bkv_sb[0:DK, 1:2])
                conv_add(k8[:, 0, QG:KG], pkv_b[0:DK, :], bkv_sb[0:DK, 0:1],
                         cfg["conv_act_upto"] >= 0)
                conv_add(k8[:, 0, 0:QG], pkv_a[0:DK, :], bkv_sb[0:DK, 0:1],
                         cfg["conv_act_upto"] >= 0)
                ps0_hi = ps_pool.tile([P, 2, QG], F32, tag="ps")
                for i in (2, 3):
                    nc.tensor.matmul(
                        ps0_hi[:, i - 2, :], k8[:, :, i * P:(i + 1) * P], qs0,
                        start=True, stop=True, perf_mode=DR,
                    )
                ps0_lo = ps_pool.tile([P, 2, QG], F32, tag="ps")
                for i in (0, 1):
                    nc.tensor.matmul(
                        ps0_lo[:, i, :], k8[:, :, i * P:(i + 1) * P], qs0,
                        start=True, stop=True, perf_mode=DR,
                    )
                ps0 = (ps0_lo, ps0_hi)
            else:
                q_proj(0)
                kv_proj(0, QG, KG)
                kv_proj(0, 0, QG)
                ps0 = ps_pool.tile([P, 4, QG], F32, tag="ps")
                for i in (2, 3, 0, 1):
                    nc.tensor.matmul(
                        ps0[:, i, :], k8[:, :, i * P:(i + 1) * P], qs0,
                        start=True, stop=True, perf_mode=DR,
                    )

            # vnat(0) is emitted inside group 0 (post fillers) so its DVE
            # copies don't queue ahead of group 1's q8 conversion

            def KP(j):
                return lambda: kv_proj(j, act=(j <= cfg["conv_act_upto"]))

            def VN(j):
                return lambda: vnat(j)

            def QP(g):
                return lambda: q_proj(g, act=(g <= cfg["conv_act_upto"]))

            nxt = ps0
            for g in range(NG):
                fillers = {}
                if g + 1 < NG:
                    qs = cfg["qp_slot"]
                    qs = qs if qs == "pre" else min(qs, g)
                    fillers.setdefault(qs, []).append(QP(g + 1))
                    if g == 0:
                        fillers["post"] = [KP(1), VN(0), VN(1)]
                    else:
                        ks, vs = min(cfg["kp_slot"], g), min(cfg["vn_slot"], g)
                        fillers.setdefault(ks, []).append(KP(g + 1))
                        fillers.setdefault(vs, []).append(VN(g + 1))
                nxt = attention(g, fillers, first_ps=nxt)

    nc.compile()
    return nc


_PROGRAM = None


def _get_program():
    global _PROGRAM
    if _PROGRAM is None:
        _PROGRAM = build_program()
    return _PROGRAM


def _host_inputs(emb, Wq_w, Wq_b, Wk_w, Wk_b, Wv_w, Wv_b):
    bf = ml_dtypes.bfloat16
    wqkv = np.zeros((E, 2 * P), np.float32)
    wqkv[:, 0:DK] = Wk_w
    wqkv[:, DK:2 * DK] = Wv_w
    wqkv[:, P:P + DK] = Wq_w
    wqkv = wqkv.astype(bf)
    wpart = wqkv.reshape(EC, P, 2 * P).transpose(1, 0, 2).reshape(P, EC * 2 * P)

    bkv = np.zeros((P, 2), np.float32)
    bkv[0:DK, 0] = Wk_b
    bkv[DK:P, 0] = Wv_b
    bkv[0:DK, 1] = Wq_b

    ones_blk = np.ones((P, P), np.float32)

    # mask for the last quad [128, 4, 256] -> flat [128, 1024]:
    # tiles 0,1: keep for role A / zero for role B; tiles 2,3: triangles
    pp = np.arange(P)[:, None]
    jj = np.arange(QG)[None, :]
    t0 = (pp <= jj).astype(np.float32)
    t1 = (pp + P <= jj).astype(np.float32)
    cst_by_role = []
    for role in range(2):
        c = np.ones((P, QG), np.float32) if role == 0 else np.zeros((P, QG), np.float32)
        bvb = np.broadcast_to(Wv_b[None, :], (P, DV))
        m = np.concatenate([c, c, t0, t1, ones_blk, bvb], axis=1).astype(bf)
        cst_by_role.append(m)
    return wpart, bkv, cst_by_role


def kernel(embedding_matrix, Wq_w, Wq_b, Wk_w, Wk_b, Wv_w, Wv_b):
    emb = np.asarray(embedding_matrix, dtype=np.float32)
    wpart, bkv, cst_by_role = _host_inputs(
        emb, np.asarray(Wq_w, np.float32), np.asarray(Wq_b, np.float32),
        np.asarray(Wk_w, np.float32), np.asarray(Wk_b, np.float32),
        np.asarray(Wv_w, np.float32), np.asarray(Wv_b, np.float32),
    )
    bf = ml_dtypes.bfloat16

    in_maps = []
    for c in range(NCORES):
        b, role = c // 2, c % 2
        e = emb[b]
        if role == 1:
            # swap the 256-halves of every 512-token block
            e = e.reshape(NJ, 2, QG, E)[:, ::-1].reshape(S, E)
        embT_sw = np.ascontiguousarray(e.T.astype(bf))
        epart = embT_sw.reshape(EC, P, S)[:, :, QG:KG]
        epart = epart.transpose(1, 0, 2).reshape(P, EC * 2 * P)
        pre = np.ascontiguousarray(np.concatenate([wpart, epart], axis=1))
        in_maps.append({
            "embT": embT_sw, "pre": pre, "bkv": bkv,
            "cst": cst_by_role[role],
        })

    nc = _get_program()
    results = run_bass_kernel_spmd(nc, in_maps, list(range(NCORES))).results

    out = np.empty((B, S, DV), np.float32)
    for c in range(NCORES):
        b, role = c // 2, c % 2
        o = results[c]["out"]                    # [NG, 256, 64]
        for g in range(NG):
            q0 = KG * g + (QG if role == 0 else 0)
            out[b, q0:q0 + QG] = o[g]
    return out


if __name__ == "__main__":
    rng = np.random.default_rng(0)
    ins = {
        "embedding_matrix": rng.standard_normal((B, S, E), dtype=np.float32),
        "Wq_w": rng.standard_normal((E, DK), dtype=np.float32) * 0.04,
        "Wq_b": rng.standard_normal((DK,), dtype=np.float32) * 0.04,
        "Wk_w": rng.standard_normal((E, DK), dtype=np.float32) * 0.04,
        "Wk_b": rng.standard_normal((DK,), dtype=np.float32) * 0.04,
        "Wv_w": rng.standard_normal((E, DV), dtype=np.float32) * 0.04,
        "Wv_b": rng.standard_normal((DV,), dtype=np.float32) * 0.04,
    }
    o = kernel(**ins)
    print("kernel ran, out:", o.shape, o.dtype, float(np.abs(o).max()))
